# revision 33
# baseline (speedup 1.0000x reference)
"""DOM pooling (segment mean+max over pulses, then linear projection) on 8 trn2 cores.

Strategy (bf16 + engine-balanced reductions):
  Host: bucket DOMs by exact pulse count k ("classes"); deal DOMs of each class
  round-robin across the 8 cores (identical class structure per core, padded
  with zero doms to an even per-core count m). On each core, consecutive doms
  are PAIRED: SBUF partition p = parity*64 + feat, so one 128-partition column
  holds one slot of two doms. Within a chunk of P dom-pairs the slot buffer is
  slot-major: col = s*P + j  (pair j, slot s), all bf16.

  Device (one NEFF, SPMD on 8 cores), per chunk:
    - DMA load (128, P*k) bf16 on the SP ring ONLY (loads have no compute
      deps -> the ring never head-of-line blocks behind compute)
    - segment-max: contiguous-halves tensor_tensor tree on DVE (bf16 2x mode)
    - segment-sum: either a DVE add-tree, or folded into the projection on the
      PE via per-slot PSUM-accumulating matmuls -- chosen per chunk to balance
      DVE vs PE time
    - projection: 128x128 block-diag matmuls (mean scale 1/k folded into the
      per-class sum weights) accumulated in PSUM, 512-col slices (1 bank each)
    - ACT adds bias during PSUM->SBUF copy (downcast bf16); weights + output
      DMA live on the ACT ring so compute-dependent stores never stall loads.

  Host: scatter per-core (128, N2) outputs back to the full (num_doms, 64).
"""
import sys

import numpy as np

for _p in ("/opt/trn_rl_repo",):
    if _p not in sys.path:
        sys.path.append(_p)

import ml_dtypes

from concourse import bacc
import concourse.mybir as mybir
import concourse.tile as tile
from concourse.bass_utils import run_bass_kernel_spmd

NCORES = 8
D = 64
FP32 = mybir.dt.float32
BF16 = mybir.dt.bfloat16
CHUNK_COLS = 16384  # max slot cols per chunk / load group (32KB/part bf16)
PMAX = 1024         # max dom-pairs per chunk (two 512-col PSUM banks)
MSLC = 512          # matmul slice width (one PSUM bank of f32)
MAXOFF = CHUNK_COLS // 4  # D-mode: max-tree offset inside the tmp tile
OBATCH = 2048       # output store batch width (cols)

# engine cost constants for load balancing (calibrated from HW traces)
DVE_COL = 0.59   # ns per 128-lane col, bf16 tensor_tensor in 2x mode
DVE_OP = 100.0   # per-op overhead (issue + drain)
PE_NS = 0.417    # ns per col, bf16 matmul stream
MM_FIX = 45.0    # per-matmul fixed (ldweights mostly hidden under streaming)
LDW_NS = 112.0   # exposed weight load when the matmul is narrow

last_exec_ns = None  # set when KERNEL_TRACE=1


def _f32_to_bf16_u16(a):
    """Round-to-nearest-even f32 -> bf16 bit pattern (uint16)."""
    u = np.ascontiguousarray(a, dtype=np.float32).view(np.uint32)
    return ((u + 0x7FFF + ((u >> 16) & 1)) >> 16).astype(np.uint16)


def _tree_cost(k, P):
    cols = 0
    ops = 0
    w = k
    while w > 1:
        h = w // 2
        cols += h * P
        ops += 1
        if w & 1:
            cols += P
            ops += 1
        w = h
    return cols * DVE_COL + ops * DVE_OP


def _mm_cost(w):
    return MM_FIX + max(w * PE_NS, LDW_NS)


def _slices(P):
    out = []
    j = 0
    while j < P:
        out.append((j, min(MSLC, P - j)))
        j += MSLC
    return out


def _plan(counts):
    """Shared class/chunk structure (identical on all cores).

    classes: (k, n_k, m, scol, ocol) ; m per-core doms (even, >= ceil(n_k/8))
    chunks:  (rank, k, c0, P, o0, eng) ; c0 slot-col offset, o0 out-col offset
    """
    kmax = int(counts.max()) if counts.size else 0
    n_k = np.bincount(counts, minlength=kmax + 1)
    classes = []
    scol = 0
    ocol = 0
    for k in range(1, kmax + 1):
        if n_k[k] == 0:
            continue
        m = -(-int(n_k[k]) // NCORES)
        m += m & 1
        classes.append((k, int(n_k[k]), m, scol, ocol))
        scol += (m // 2) * k
        ocol += m // 2
    S, N2 = scol, ocol

    raw = []
    for rank, (k, nk, m, sc, oc) in enumerate(classes):
        P_k = max(1, min(PMAX, CHUNK_COLS // k))
        pairs = m // 2
        j = 0
        while j < pairs:
            P = min(P_k, pairs - j)
            raw.append((rank, k, j, P, oc + j))
            j += P
    # engine assignment (greedy balance), biggest chunks first
    by_cols = sorted(range(len(raw)), key=lambda i: -raw[i][1] * raw[i][3])
    eng_of = {}
    dve = pe = 0.0
    for i in by_cols:
        rank, k, c0, P, o0 = raw[i]
        sl = _slices(P)
        if k == 1:
            eng_of[i] = "-"
            pe += sum(_mm_cost(w) for _, w in sl)
            continue
        tcost = _tree_cost(k, P)
        dve += tcost  # max tree always on DVE
        d_pe = sum(2 * _mm_cost(w) for _, w in sl)
        p_pe = sum((k + 1) * _mm_cost(w) for _, w in sl)
        # D-mode keeps both trees in one tmp tile: level-0 extent must fit
        # half of it (the group input tile itself stays read-only)
        d_ok = (k // 2) * P <= CHUNK_COLS // 4
        if d_ok and (
            (k * P < 256 and k >= 6)
            or max(dve + tcost, pe + d_pe) <= max(dve, pe + p_pe)
        ):
            eng_of[i] = "D"
            dve += tcost
            pe += d_pe
        else:
            eng_of[i] = "P"
            pe += p_pe
    # emission order. Slot/out offsets are absolute, so processing order is
    # free. Goals: (a) small chunks first for fast pipeline fill; (b) weave
    # DVE-heavy D chunks between P chunks so the DVE never falls behind the
    # load stream for longer than the input-buffer lookahead; (c) end with
    # the chunks whose post-load compute chain (the DVE tree) is shortest,
    # k==1 (no DVE at all) last, so the drain tail after the final load is
    # minimal.
    def drain(i):
        rank, k, c0, P, o0 = raw[i]
        if k == 1:
            return 0.0
        t = _tree_cost(k, P)
        if eng_of[i] == "D":
            return 2 * t
        return max(t, (k + 1) * sum(_mm_cost(w) for _, w in _slices(P)))
    k1 = [i for i in range(len(raw)) if raw[i][1] == 1]
    rest = [i for i in range(len(raw)) if raw[i][1] != 1]
    rest.sort(key=lambda i: raw[i][1] * raw[i][3])
    # head: the two smallest non-trivial chunks (>=1024 cols) -- big enough
    # to prime the DVE/PE pipeline, small enough to load fast
    head = [i for i in rest if raw[i][1] * raw[i][3] >= 1024][:2]
    rest = [i for i in rest if i not in head]
    tail = sorted(rest, key=drain)[:3]
    rest = [i for i in rest if i not in tail]
    dlist = [i for i in rest if eng_of[i] == "D"]
    plist = [i for i in rest if eng_of[i] == "P"]
    dlist.sort(key=lambda i: -raw[i][1] * raw[i][3])
    plist.sort(key=lambda i: -raw[i][1] * raw[i][3])
    # micro chunks carry only fixed costs -- tuck them mid-stream, not at
    # the end where their per-op overhead would sit on the drain path
    micro = [i for i in plist if raw[i][1] * raw[i][3] < 512]
    if micro:
        bigs = [i for i in plist if i not in micro]
        plist = bigs[: len(bigs) // 2] + micro + bigs[len(bigs) // 2 :]
    mid = []
    di = pi = 0
    stride = max(1, (len(plist) + len(dlist) - 1) // max(1, len(dlist)))
    while pi < len(plist) or di < len(dlist):
        take_p = min(stride - 1, len(plist) - pi)
        mid.extend(plist[pi : pi + take_p])
        pi += take_p
        if di < len(dlist):
            mid.append(dlist[di])
            di += 1
        if pi >= len(plist) and di >= len(dlist):
            break
        if take_p == 0 and di >= len(dlist):
            mid.extend(plist[pi:])
            break
    order = head + mid + sorted(tail, key=lambda i: -drain(i)) + k1
    assert sorted(order) == list(range(len(raw)))
    # slot-buffer DRAM layout and output columns follow emission order (the
    # host writes blocks / scatters results accordingly), so loads consume a
    # single contiguous cursor and consecutive chunks' outputs are adjacent
    # (-> stores can batch)
    chunks = []
    cur = 0
    ocur = 0
    for i in order:
        rank, k, j0, P, _ = raw[i]
        chunks.append((rank, k, cur, P, ocur, eng_of[i], j0))
        cur += k * P
        ocur += P
    assert cur == S and ocur == N2
    # pack chunks into contiguous load groups: one dma_start + one SBUF tile
    # per group, so tiny chunks never collapse the in-flight byte lookahead.
    # Small groups at the head (fast pipeline fill) and tail (fine-grained
    # drain), full-size in the middle.
    groups = []
    g = []
    gcols = 0
    for ch in chunks:
        cols = ch[1] * ch[3]
        c_end = ch[2] + cols
        if c_end <= 4096:
            tgt = 2048
        elif c_end <= 16384:
            tgt = 8192
        elif c_end >= S - 8192:
            tgt = 2048
        elif c_end >= S - 32768:
            tgt = 8192
        else:
            tgt = CHUNK_COLS
        if g and gcols + cols > tgt:
            groups.append(g)
            g = []
            gcols = 0
        g.append(ch)
        gcols += cols
    if g:
        groups.append(g)
    return classes, groups, S, N2


def _build_nc(classes, groups, S, N2):
    nblk = len(classes) + 1  # per-class sum blocks + shared max block
    jmax = len(classes)

    nc = bacc.Bacc(None)
    slots_t = nc.dram_tensor("slots", [128, S], BF16, kind="ExternalInput")
    # weights pre-transposed on host to the SBUF layout (one clean DMA; a
    # compact form with strided expansion stalls the ACT ring for ~23us of
    # descriptor generation -- measured, not worth the 0.95MB saving)
    wts_t = nc.dram_tensor("wts", [128, nblk * 128], BF16, kind="ExternalInput")
    b_t = nc.dram_tensor("b", [128, 1], FP32, kind="ExternalInput")
    out_t = nc.dram_tensor("out", [128, N2], BF16, kind="ExternalOutput")

    ADD = mybir.AluOpType.add
    MAX = mybir.AluOpType.max

    def emit_tree(eng, dst, src, k, P, op):
        """Reduce k slot-major blocks of P cols: result lands in dst[:, :P].

        Level 0 reads src, writes dst (dst may be src for in-place); later
        levels run in-place on dst. Contiguous operands keep DVE 2x mode.
        """
        w = k
        first = True
        while w > 1:
            h = w // 2
            a = dst if not first else src
            eng.tensor_tensor(
                out=dst[:, : h * P], in0=a[:, : h * P],
                in1=a[:, h * P : 2 * h * P], op=op,
            )
            if w & 1:
                eng.tensor_tensor(
                    out=dst[:, (h - 1) * P : h * P],
                    in0=dst[:, (h - 1) * P : h * P],
                    in1=a[:, 2 * h * P : (2 * h + 1) * P], op=op,
                )
            w = h
            first = False

    with tile.TileContext(nc) as tc:
        with (
            tc.tile_pool(name="const", bufs=1) as constp,
            tc.tile_pool(name="inp", bufs=4) as inp,
            tc.tile_pool(name="tmpp", bufs=3) as tmpp,
            tc.tile_pool(name="outp", bufs=4) as outp,
            tc.tile_pool(name="psp", bufs=8, space="PSUM") as psp,
        ):
            # weights/bias on the ACT ring (stores also live there; the SP
            # ring carries nothing but input loads)
            wt_sb = constp.tile([128, nblk * 128], BF16)
            nc.scalar.dma_start(wt_sb[:], wts_t[:, :])
            b_sb = constp.tile([128, 1], FP32)
            nc.scalar.dma_start(b_sb[:], b_t[:])

            # output batching: chunk outputs are emission-adjacent, so ACT
            # results accumulate in one SBUF tile and store as a single DMA
            ob = {"tile": None, "start": 0, "fill": 0}

            def ob_close():
                if ob["tile"] is not None and ob["fill"]:
                    nc.scalar.dma_start(
                        out_t[:, ob["start"] : ob["start"] + ob["fill"]],
                        ob["tile"][:, : ob["fill"]],
                    )
                ob["tile"] = None
                ob["fill"] = 0

            def flush(st):
                """Emit the DVE-dependent matmuls + ACT + out-DMA of a chunk.

                Deferred one chunk so the PE has the next chunk's independent
                sum matmuls to chew on while the DVE tree finishes (PSUM
                groups interleave across chunks -> skip_group_check).
                """
                rank, k, P, o0, eng, pss, in_t, tmp = st
                if ob["tile"] is None or ob["fill"] + P > OBATCH:
                    ob_close()
                    ob["tile"] = outp.tile([128, OBATCH], BF16, tag="out",
                                           name="out_sb")
                    ob["start"] = o0
                out_sb = ob["tile"][:, ob["fill"] : ob["fill"] + P]
                ob["fill"] += P
                for (j, w), ps in zip(_slices(P), pss):
                    if eng != "P":
                        src = tmp if eng == "D" else in_t
                        nc.tensor.matmul(
                            ps[:, :w],
                            lhsT=wt_sb[:, rank * 128 : (rank + 1) * 128],
                            rhs=src[:, j : j + w],
                            start=True, stop=(k == 1),
                            skip_group_check=True,
                        )
                    if k != 1:
                        # max-tree result: offset 0 in tmp for P-mode, upper
                        # half of tmp for D-mode (sum tree owns the lower)
                        moff = MAXOFF if eng == "D" else 0
                        nc.tensor.matmul(
                            ps[:, :w],
                            lhsT=wt_sb[:, jmax * 128 : (jmax + 1) * 128],
                            rhs=tmp[:, moff + j : moff + j + w],
                            start=False, stop=True,
                            skip_group_check=True,
                        )
                    nc.scalar.activation(
                        out_sb[:, j : j + w], ps[:, :w],
                        mybir.ActivationFunctionType.Identity, bias=b_sb[:, :1],
                    )

            pending = None
            for grp in groups:
                g0 = grp[0][2]
                gcols = sum(k * P for _, k, _, P, _, _, _ in grp)
                gr_t = inp.tile([128, CHUNK_COLS], BF16, tag="in")
                nc.sync.dma_start(gr_t[:, :gcols], slots_t[:, g0 : g0 + gcols])
                for rank, k, c0, P, o0, eng, _ in grp:
                    in_t = gr_t[:, c0 - g0 : c0 - g0 + k * P]
                    pss = [psp.tile([128, MSLC], FP32, space="PSUM", tag="ps",
                                    name="ps")
                           for _ in _slices(P)]
                    tmp = None
                    if k == 1:
                        pass  # single combined matmul, emitted in flush
                    elif eng == "P":
                        # max tree on DVE (non-destructive, into tmp); sum on
                        # PE via per-slot PSUM-accumulating matmuls (no DVE
                        # dep)
                        tmp = tmpp.tile([128, CHUNK_COLS // 2], BF16,
                                        tag="tmp")
                        emit_tree(nc.vector, tmp, in_t, k, P, MAX)
                        for s in range(k):
                            for (j, w), ps in zip(_slices(P), pss):
                                nc.tensor.matmul(
                                    ps[:, :w],
                                    lhsT=wt_sb[:, rank * 128 : (rank + 1) * 128],
                                    rhs=in_t[:, s * P + j : s * P + j + w],
                                    start=(s == 0), stop=False,
                                    skip_group_check=True,
                                )
                    else:
                        # both trees on DVE, side by side in one tmp tile
                        # (the group input tile stays read-only)
                        tmp = tmpp.tile([128, CHUNK_COLS // 2], BF16,
                                        tag="tmp")
                        emit_tree(nc.vector, tmp[:, :MAXOFF], in_t, k, P, ADD)
                        emit_tree(nc.vector, tmp[:, MAXOFF:], in_t, k, P, MAX)
                    if pending is not None:
                        flush(pending)
                    pending = (rank, k, P, o0, eng, pss, in_t, tmp)
            if pending is not None:
                flush(pending)
            ob_close()
    nc.finalize()
    return nc


def kernel(pulse_embeddings, pulse_to_dom_idx, num_doms, proj_w, proj_b):
    global last_exec_ns
    import os

    E = np.ascontiguousarray(np.asarray(pulse_embeddings, dtype=np.float32))
    idx = np.asarray(pulse_to_dom_idx).astype(np.int64)
    nd = int(num_doms)
    W = np.asarray(proj_w, dtype=np.float32)   # (D, 2D)
    b = np.asarray(proj_b, dtype=np.float32)   # (D,)
    NP = E.shape[0]

    counts = np.bincount(idx, minlength=nd)
    classes, groups, S, N2 = _plan(counts)
    chunks = [ch for g in groups for ch in g]

    # ---- dom assignment --------------------------------------------------
    dom_order = np.argsort(counts, kind="stable")
    n0 = int((counts == 0).sum())
    dom_class = np.full(nd, -1, np.int32)
    dom_core = np.zeros(nd, np.int8)
    dom_pos = np.zeros(nd, np.int32)
    off = n0
    for rank, (k, nk, m, sc, oc) in enumerate(classes):
        doms = dom_order[off : off + nk]
        off += nk
        ar = np.arange(nk, dtype=np.int64)
        dom_class[doms] = rank
        dom_core[doms] = ar % NCORES
        dom_pos[doms] = ar // NCORES

    # pulses grouped by (core, class, pos); within a dom original order
    dom_key = (
        (dom_core.astype(np.int64) << 40)
        | (dom_class.astype(np.int64) << 20)
        | dom_pos.astype(np.int64)
    )
    perm = np.argsort(dom_key[idx], kind="stable").astype(np.int32)

    # pulse count per (core, class): n_c * k
    core_cls_pulses = np.zeros((NCORES, len(classes)), np.int64)
    for rank, (k, nk, m, sc, oc) in enumerate(classes):
        n_c = nk // NCORES + (np.arange(NCORES) < nk % NCORES)
        core_cls_pulses[:, rank] = n_c * k
    core_off = np.concatenate([[0], np.cumsum(core_cls_pulses.sum(axis=1))])

    # ---- slot buffers ----------------------------------------------------
    Eb = _f32_to_bf16_u16(E)                      # (NP, 64) uint16
    E2b = np.vstack([Eb, np.zeros((1, D), np.uint16)])
    Z = NP

    bufs = []
    for c in range(NCORES):
        R2s = []
        p_off = int(core_off[c])
        for rank, (k, nk, m, sc, oc) in enumerate(classes):
            n_c = nk // NCORES + (1 if c < nk % NCORES else 0)
            R = np.full((m, k), Z, np.int32)
            if n_c:
                R[:n_c] = perm[p_off : p_off + n_c * k].reshape(n_c, k)
                p_off += n_c * k
            R2s.append(R.reshape(m // 2, 2, k))
        # blocks written in chunk-emission order: loads consume the slot
        # buffer as one contiguous cursor
        blocks = []
        for rank, k, c0, P, o0, eng, j0 in chunks:
            blk = R2s[rank][j0 : j0 + P]                  # (P, 2, k)
            blocks.append(blk.transpose(1, 2, 0).reshape(2, k * P))
        ridx = np.concatenate(blocks, axis=1)             # (2, S)
        g = E2b[ridx]                                     # (2, S, 64) uint16
        buf = np.ascontiguousarray(g.transpose(0, 2, 1)).reshape(128, S)
        bufs.append(buf.view(ml_dtypes.bfloat16))

    # ---- weights / bias --------------------------------------------------
    Wsum = W[:, :D]
    Wmax = W[:, D:]

    def blkdiag(M):
        Z2 = np.zeros((128, 128), np.float32)
        Z2[:D, :D] = M
        Z2[D:, D:] = M
        return Z2

    wblocks = []
    for rank, (k, nk, m, sc, oc) in enumerate(classes):
        if k == 1:
            wblocks.append(blkdiag((Wsum + Wmax).T))
        else:
            wblocks.append(blkdiag(Wsum.T / np.float32(k)))
    wblocks.append(blkdiag(Wmax.T))
    # (nblk*128, 128) -> SBUF layout (128, nblk*128): partition p, col j*128+e
    wcat = np.concatenate(wblocks, axis=0).reshape(-1, 128, 128)
    wcat = np.ascontiguousarray(wcat.transpose(1, 0, 2)).reshape(128, -1)
    wts = _f32_to_bf16_u16(wcat).view(ml_dtypes.bfloat16)
    b128 = np.concatenate([b, b]).reshape(128, 1).astype(np.float32)

    # ---- device ----------------------------------------------------------
    nc = _build_nc(classes, groups, S, N2)
    in_maps = [{"slots": bufs[c], "wts": wts, "b": b128} for c in range(NCORES)]
    trace = os.environ.get("KERNEL_TRACE", "0") == "1"
    kw_ = {}
    if trace:
        import tempfile
        kw_ = dict(trace=True, tmpdir=tempfile.mkdtemp(prefix="kernel_trace_"))
    res = run_bass_kernel_spmd(nc, in_maps, core_ids=list(range(NCORES)), **kw_)
    last_exec_ns = res.exec_time_ns

    # ---- scatter back ----------------------------------------------------
    outs = np.stack(
        [np.asarray(res.results[c]["out"], dtype=np.float32) for c in range(NCORES)]
    )                                                     # (8, 128, N2)
    outs = outs.reshape(NCORES, 2, D, N2)
    # per-(class, pair) output column (chunks relabel o0 in emission order)
    cls_base = np.concatenate(
        [[0], np.cumsum([m // 2 for (k, nk, m, sc, oc) in classes])]
    ).astype(np.int64)
    pair_ocol = np.zeros(int(cls_base[-1]), np.int64)
    for rank, k, c0, P, o0, eng, j0 in chunks:
        pair_ocol[cls_base[rank] + j0 : cls_base[rank] + j0 + P] = o0 + np.arange(P)
    real = counts > 0
    d_core = dom_core[real].astype(np.int64)
    d_ocol = pair_ocol[cls_base[dom_class[real]] + dom_pos[real] // 2]
    d_par = dom_pos[real] % 2
    full = np.empty((nd, D), np.float32)
    full[real] = outs[d_core, d_par, :, d_ocol]
    full[~real] = b
    return full


# revision 38
# speedup vs baseline: 1.0833x; 1.0833x over previous
"""DOM pooling (segment mean+max over pulses, then linear projection) on 8 trn2 cores.

Strategy (bf16 + engine-balanced reductions):
  Host: bucket DOMs by exact pulse count k ("classes"); deal DOMs of each class
  round-robin across the 8 cores (identical class structure per core, padded
  with zero doms to an even per-core count m). On each core, consecutive doms
  are PAIRED: SBUF partition p = parity*64 + feat, so one 128-partition column
  holds one slot of two doms. Within a chunk of P dom-pairs the slot buffer is
  slot-major: col = s*P + j  (pair j, slot s), all bf16.

  Device (one NEFF, SPMD on 8 cores), per chunk:
    - DMA load (128, P*k) bf16 on the SP ring ONLY (loads have no compute
      deps -> the ring never head-of-line blocks behind compute)
    - segment-max: contiguous-halves tensor_tensor tree on DVE (bf16 2x mode)
    - segment-sum: either a DVE add-tree, or folded into the projection on the
      PE via per-slot PSUM-accumulating matmuls -- chosen per chunk to balance
      DVE vs PE time
    - projection: 128x128 block-diag matmuls (mean scale 1/k folded into the
      per-class sum weights) accumulated in PSUM, 512-col slices (1 bank each)
    - ACT adds bias during PSUM->SBUF copy (downcast bf16); weights + output
      DMA live on the ACT ring so compute-dependent stores never stall loads.

  Host: scatter per-core (128, N2) outputs back to the full (num_doms, 64).
"""
import sys

import numpy as np

for _p in ("/opt/trn_rl_repo",):
    if _p not in sys.path:
        sys.path.append(_p)

import ml_dtypes

from concourse import bacc
import concourse.mybir as mybir
import concourse.tile as tile
from concourse.bass_utils import run_bass_kernel_spmd

NCORES = 8
D = 64
FP32 = mybir.dt.float32
BF16 = mybir.dt.bfloat16
CHUNK_COLS = 16384  # max slot cols per chunk / load group (32KB/part bf16)
PMAX = 1024         # max dom-pairs per chunk (two 512-col PSUM banks)
MSLC = 512          # matmul slice width (one PSUM bank of f32)
MAXOFF = CHUNK_COLS // 4  # D-mode: max-tree offset inside the tmp tile

# engine cost constants for load balancing (calibrated from HW traces)
DVE_COL = 0.59   # ns per 128-lane col, bf16 tensor_tensor in 2x mode
DVE_OP = 100.0   # per-op overhead (issue + drain)
PE_NS = 0.417    # ns per col, bf16 matmul stream
MM_FIX = 45.0    # per-matmul fixed (ldweights mostly hidden under streaming)
LDW_NS = 112.0   # exposed weight load when the matmul is narrow

last_exec_ns = None  # set when KERNEL_TRACE=1


def _f32_to_bf16_u16(a):
    """Round-to-nearest-even f32 -> bf16 bit pattern (uint16)."""
    u = np.ascontiguousarray(a, dtype=np.float32).view(np.uint32)
    return ((u + 0x7FFF + ((u >> 16) & 1)) >> 16).astype(np.uint16)


def _tree_cost(k, P):
    cols = 0
    ops = 0
    w = k
    while w > 1:
        h = w // 2
        cols += h * P
        ops += 1
        if w & 1:
            cols += P
            ops += 1
        w = h
    return cols * DVE_COL + ops * DVE_OP


def _mm_cost(w):
    return MM_FIX + max(w * PE_NS, LDW_NS)


def _slices(P):
    out = []
    j = 0
    while j < P:
        out.append((j, min(MSLC, P - j)))
        j += MSLC
    return out


def _plan(counts):
    """Shared class/chunk structure (identical on all cores).

    classes: (k, n_k, m, scol, ocol) ; m per-core doms (even, >= ceil(n_k/8))
    chunks:  (rank, k, c0, P, o0, eng) ; c0 slot-col offset, o0 out-col offset
    """
    kmax = int(counts.max()) if counts.size else 0
    n_k = np.bincount(counts, minlength=kmax + 1)
    classes = []
    scol = 0
    ocol = 0
    for k in range(1, kmax + 1):
        if n_k[k] == 0:
            continue
        m = -(-int(n_k[k]) // NCORES)
        m += m & 1
        classes.append((k, int(n_k[k]), m, scol, ocol))
        scol += (m // 2) * k
        ocol += m // 2
    S, N2 = scol, ocol

    raw = []
    for rank, (k, nk, m, sc, oc) in enumerate(classes):
        P_k = max(1, min(PMAX, CHUNK_COLS // k))
        pairs = m // 2
        j = 0
        while j < pairs:
            P = min(P_k, pairs - j)
            raw.append((rank, k, j, P, oc + j))
            j += P
    # engine assignment (greedy balance), biggest chunks first
    by_cols = sorted(range(len(raw)), key=lambda i: -raw[i][1] * raw[i][3])
    eng_of = {}
    dve = pe = 0.0
    for i in by_cols:
        rank, k, c0, P, o0 = raw[i]
        sl = _slices(P)
        if k == 1:
            eng_of[i] = "-"
            pe += sum(_mm_cost(w) for _, w in sl)
            continue
        tcost = _tree_cost(k, P)
        dve += tcost  # max tree always on DVE
        d_pe = sum(2 * _mm_cost(w) for _, w in sl)
        p_pe = sum((k + 1) * _mm_cost(w) for _, w in sl)
        # D-mode keeps both trees in one tmp tile: level-0 extent must fit
        # half of it (the group input tile itself stays read-only)
        d_ok = (k // 2) * P <= CHUNK_COLS // 4
        if d_ok and (
            (k * P < 256 and k >= 6)
            or max(dve + tcost, pe + d_pe) <= max(dve, pe + p_pe)
        ):
            eng_of[i] = "D"
            dve += tcost
            pe += d_pe
        else:
            eng_of[i] = "P"
            pe += p_pe
    # emission order. Slot/out offsets are absolute, so processing order is
    # free. Goals: (a) small chunks first for fast pipeline fill; (b) weave
    # DVE-heavy D chunks between P chunks so the DVE never falls behind the
    # load stream for longer than the input-buffer lookahead; (c) end with
    # the chunks whose post-load compute chain (the DVE tree) is shortest,
    # k==1 (no DVE at all) last, so the drain tail after the final load is
    # minimal.
    def drain(i):
        rank, k, c0, P, o0 = raw[i]
        if k == 1:
            return 0.0
        t = _tree_cost(k, P)
        if eng_of[i] == "D":
            return 2 * t
        return max(t, (k + 1) * sum(_mm_cost(w) for _, w in _slices(P)))
    k1 = [i for i in range(len(raw)) if raw[i][1] == 1]
    rest = [i for i in range(len(raw)) if raw[i][1] != 1]
    rest.sort(key=lambda i: raw[i][1] * raw[i][3])
    # head: the two smallest non-trivial chunks (>=1024 cols) -- big enough
    # to prime the DVE/PE pipeline, small enough to load fast
    head = [i for i in rest if raw[i][1] * raw[i][3] >= 1024][:2]
    rest = [i for i in rest if i not in head]
    tail = sorted(rest, key=drain)[:3]
    rest = [i for i in rest if i not in tail]
    dlist = [i for i in rest if eng_of[i] == "D"]
    plist = [i for i in rest if eng_of[i] == "P"]
    dlist.sort(key=lambda i: -raw[i][1] * raw[i][3])
    plist.sort(key=lambda i: -raw[i][1] * raw[i][3])
    # micro chunks carry only fixed costs -- tuck them mid-stream, not at
    # the end where their per-op overhead would sit on the drain path
    micro = [i for i in plist if raw[i][1] * raw[i][3] < 512]
    if micro:
        bigs = [i for i in plist if i not in micro]
        plist = bigs[: len(bigs) // 2] + micro + bigs[len(bigs) // 2 :]
    mid = []
    di = pi = 0
    stride = max(1, (len(plist) + len(dlist) - 1) // max(1, len(dlist)))
    while pi < len(plist) or di < len(dlist):
        take_p = min(stride - 1, len(plist) - pi)
        mid.extend(plist[pi : pi + take_p])
        pi += take_p
        if di < len(dlist):
            mid.append(dlist[di])
            di += 1
        if pi >= len(plist) and di >= len(dlist):
            break
        if take_p == 0 and di >= len(dlist):
            mid.extend(plist[pi:])
            break
    order = head + mid + sorted(tail, key=lambda i: -drain(i)) + k1
    assert sorted(order) == list(range(len(raw)))
    # slot-buffer DRAM layout and output columns follow emission order (the
    # host writes blocks / scatters results accordingly), so loads consume a
    # single contiguous cursor and consecutive chunks' outputs are adjacent
    # (-> stores can batch)
    chunks = []
    cur = 0
    ocur = 0
    for i in order:
        rank, k, j0, P, _ = raw[i]
        chunks.append((rank, k, cur, P, ocur, eng_of[i], j0))
        cur += k * P
        ocur += P
    assert cur == S and ocur == N2
    # pack chunks into contiguous load groups: one dma_start + one SBUF tile
    # per group, so tiny chunks never collapse the in-flight byte lookahead.
    # Small groups at the head (fast pipeline fill) and tail (fine-grained
    # drain), full-size in the middle.
    groups = []
    g = []
    gcols = 0
    for ch in chunks:
        cols = ch[1] * ch[3]
        c_end = ch[2] + cols
        if c_end <= 4096:
            tgt = 2048
        elif c_end <= 16384:
            tgt = 8192
        elif c_end >= S - 24576:
            tgt = 8192
        else:
            tgt = CHUNK_COLS
        if g and gcols + cols > tgt:
            groups.append(g)
            g = []
            gcols = 0
        g.append(ch)
        gcols += cols
    if g:
        groups.append(g)
    return classes, groups, S, N2


def _build_nc(classes, groups, S, N2):
    nblk = len(classes) + 1  # per-class sum blocks + shared max block
    jmax = len(classes)

    nc = bacc.Bacc(None)
    slots_t = nc.dram_tensor("slots", [128, S], BF16, kind="ExternalInput")
    # weights pre-transposed on host to the SBUF layout (one clean DMA; a
    # compact form with strided expansion stalls the ACT ring for ~23us of
    # descriptor generation -- measured, not worth the 0.95MB saving)
    wts_t = nc.dram_tensor("wts", [128, nblk * 128], BF16, kind="ExternalInput")
    b_t = nc.dram_tensor("b", [128, 1], FP32, kind="ExternalInput")
    out_t = nc.dram_tensor("out", [128, N2], BF16, kind="ExternalOutput")

    ADD = mybir.AluOpType.add
    MAX = mybir.AluOpType.max

    def emit_tree(eng, dst, src, k, P, op):
        """Reduce k slot-major blocks of P cols: result lands in dst[:, :P].

        Level 0 reads src, writes dst (dst may be src for in-place); later
        levels run in-place on dst. Contiguous operands keep DVE 2x mode.
        """
        w = k
        first = True
        while w > 1:
            h = w // 2
            a = dst if not first else src
            eng.tensor_tensor(
                out=dst[:, : h * P], in0=a[:, : h * P],
                in1=a[:, h * P : 2 * h * P], op=op,
            )
            if w & 1:
                eng.tensor_tensor(
                    out=dst[:, (h - 1) * P : h * P],
                    in0=dst[:, (h - 1) * P : h * P],
                    in1=a[:, 2 * h * P : (2 * h + 1) * P], op=op,
                )
            w = h
            first = False

    with tile.TileContext(nc) as tc:
        with (
            tc.tile_pool(name="const", bufs=1) as constp,
            tc.tile_pool(name="inp", bufs=4) as inp,
            tc.tile_pool(name="tmpp", bufs=3) as tmpp,
            tc.tile_pool(name="outp", bufs=4) as outp,
            tc.tile_pool(name="psp", bufs=8, space="PSUM") as psp,
        ):
            # weights/bias on the ACT ring (stores also live there; the SP
            # ring carries nothing but input loads)
            wt_sb = constp.tile([128, nblk * 128], BF16)
            nc.scalar.dma_start(wt_sb[:], wts_t[:, :])
            b_sb = constp.tile([128, 1], FP32)
            nc.scalar.dma_start(b_sb[:], b_t[:])

            def flush(st):
                """Emit the DVE-dependent matmuls + ACT + out-DMA of a chunk.

                Deferred one chunk so the PE has the next chunk's independent
                sum matmuls to chew on while the DVE tree finishes (PSUM
                groups interleave across chunks -> skip_group_check).
                """
                rank, k, P, o0, eng, pss, in_t, tmp = st
                out_sb = outp.tile([128, PMAX], BF16, tag="out")
                for (j, w), ps in zip(_slices(P), pss):
                    if eng != "P":
                        src = tmp if eng == "D" else in_t
                        nc.tensor.matmul(
                            ps[:, :w],
                            lhsT=wt_sb[:, rank * 128 : (rank + 1) * 128],
                            rhs=src[:, j : j + w],
                            start=True, stop=(k == 1),
                            skip_group_check=True,
                        )
                    if k != 1:
                        # max-tree result: offset 0 in tmp for P-mode, upper
                        # half of tmp for D-mode (sum tree owns the lower)
                        moff = MAXOFF if eng == "D" else 0
                        nc.tensor.matmul(
                            ps[:, :w],
                            lhsT=wt_sb[:, jmax * 128 : (jmax + 1) * 128],
                            rhs=tmp[:, moff + j : moff + j + w],
                            start=False, stop=True,
                            skip_group_check=True,
                        )
                    nc.scalar.activation(
                        out_sb[:, j : j + w], ps[:, :w],
                        mybir.ActivationFunctionType.Identity, bias=b_sb[:, :1],
                    )
                nc.scalar.dma_start(out_t[:, o0 : o0 + P], out_sb[:, :P])

            pending = None
            for grp in groups:
                g0 = grp[0][2]
                gcols = sum(k * P for _, k, _, P, _, _, _ in grp)
                gr_t = inp.tile([128, CHUNK_COLS], BF16, tag="in")
                nc.sync.dma_start(gr_t[:, :gcols], slots_t[:, g0 : g0 + gcols])
                for rank, k, c0, P, o0, eng, _ in grp:
                    in_t = gr_t[:, c0 - g0 : c0 - g0 + k * P]
                    pss = [psp.tile([128, MSLC], FP32, space="PSUM", tag="ps",
                                    name="ps")
                           for _ in _slices(P)]
                    tmp = None
                    if k == 1:
                        pass  # single combined matmul, emitted in flush
                    elif eng == "P":
                        # max tree on DVE (non-destructive, into tmp); sum on
                        # PE via per-slot PSUM-accumulating matmuls (no DVE
                        # dep)
                        tmp = tmpp.tile([128, CHUNK_COLS // 2], BF16,
                                        tag="tmp")
                        emit_tree(nc.vector, tmp, in_t, k, P, MAX)
                        for s in range(k):
                            for (j, w), ps in zip(_slices(P), pss):
                                nc.tensor.matmul(
                                    ps[:, :w],
                                    lhsT=wt_sb[:, rank * 128 : (rank + 1) * 128],
                                    rhs=in_t[:, s * P + j : s * P + j + w],
                                    start=(s == 0), stop=False,
                                    skip_group_check=True,
                                )
                    else:
                        # both trees on DVE, side by side in one tmp tile
                        # (the group input tile stays read-only)
                        tmp = tmpp.tile([128, CHUNK_COLS // 2], BF16,
                                        tag="tmp")
                        emit_tree(nc.vector, tmp[:, :MAXOFF], in_t, k, P, ADD)
                        emit_tree(nc.vector, tmp[:, MAXOFF:], in_t, k, P, MAX)
                    if pending is not None:
                        flush(pending)
                    pending = (rank, k, P, o0, eng, pss, in_t, tmp)
            if pending is not None:
                flush(pending)
    nc.finalize()
    return nc


def kernel(pulse_embeddings, pulse_to_dom_idx, num_doms, proj_w, proj_b):
    global last_exec_ns
    import os

    E = np.ascontiguousarray(np.asarray(pulse_embeddings, dtype=np.float32))
    idx = np.asarray(pulse_to_dom_idx).astype(np.int64)
    nd = int(num_doms)
    W = np.asarray(proj_w, dtype=np.float32)   # (D, 2D)
    b = np.asarray(proj_b, dtype=np.float32)   # (D,)
    NP = E.shape[0]

    counts = np.bincount(idx, minlength=nd)
    classes, groups, S, N2 = _plan(counts)
    chunks = [ch for g in groups for ch in g]

    # ---- dom assignment --------------------------------------------------
    dom_order = np.argsort(counts, kind="stable")
    n0 = int((counts == 0).sum())
    dom_class = np.full(nd, -1, np.int32)
    dom_core = np.zeros(nd, np.int8)
    dom_pos = np.zeros(nd, np.int32)
    off = n0
    for rank, (k, nk, m, sc, oc) in enumerate(classes):
        doms = dom_order[off : off + nk]
        off += nk
        ar = np.arange(nk, dtype=np.int64)
        dom_class[doms] = rank
        dom_core[doms] = ar % NCORES
        dom_pos[doms] = ar // NCORES

    # pulses grouped by (core, class, pos); within a dom original order
    dom_key = (
        (dom_core.astype(np.int64) << 40)
        | (dom_class.astype(np.int64) << 20)
        | dom_pos.astype(np.int64)
    )
    perm = np.argsort(dom_key[idx], kind="stable").astype(np.int32)

    # pulse count per (core, class): n_c * k
    core_cls_pulses = np.zeros((NCORES, len(classes)), np.int64)
    for rank, (k, nk, m, sc, oc) in enumerate(classes):
        n_c = nk // NCORES + (np.arange(NCORES) < nk % NCORES)
        core_cls_pulses[:, rank] = n_c * k
    core_off = np.concatenate([[0], np.cumsum(core_cls_pulses.sum(axis=1))])

    # ---- slot buffers ----------------------------------------------------
    Eb = _f32_to_bf16_u16(E)                      # (NP, 64) uint16
    E2b = np.vstack([Eb, np.zeros((1, D), np.uint16)])
    Z = NP

    bufs = []
    for c in range(NCORES):
        R2s = []
        p_off = int(core_off[c])
        for rank, (k, nk, m, sc, oc) in enumerate(classes):
            n_c = nk // NCORES + (1 if c < nk % NCORES else 0)
            R = np.full((m, k), Z, np.int32)
            if n_c:
                R[:n_c] = perm[p_off : p_off + n_c * k].reshape(n_c, k)
                p_off += n_c * k
            R2s.append(R.reshape(m // 2, 2, k))
        # blocks written in chunk-emission order: loads consume the slot
        # buffer as one contiguous cursor
        blocks = []
        for rank, k, c0, P, o0, eng, j0 in chunks:
            blk = R2s[rank][j0 : j0 + P]                  # (P, 2, k)
            blocks.append(blk.transpose(1, 2, 0).reshape(2, k * P))
        ridx = np.concatenate(blocks, axis=1)             # (2, S)
        g = E2b[ridx]                                     # (2, S, 64) uint16
        buf = np.ascontiguousarray(g.transpose(0, 2, 1)).reshape(128, S)
        bufs.append(buf.view(ml_dtypes.bfloat16))

    # ---- weights / bias --------------------------------------------------
    Wsum = W[:, :D]
    Wmax = W[:, D:]

    def blkdiag(M):
        Z2 = np.zeros((128, 128), np.float32)
        Z2[:D, :D] = M
        Z2[D:, D:] = M
        return Z2

    wblocks = []
    for rank, (k, nk, m, sc, oc) in enumerate(classes):
        if k == 1:
            wblocks.append(blkdiag((Wsum + Wmax).T))
        else:
            wblocks.append(blkdiag(Wsum.T / np.float32(k)))
    wblocks.append(blkdiag(Wmax.T))
    # (nblk*128, 128) -> SBUF layout (128, nblk*128): partition p, col j*128+e
    wcat = np.concatenate(wblocks, axis=0).reshape(-1, 128, 128)
    wcat = np.ascontiguousarray(wcat.transpose(1, 0, 2)).reshape(128, -1)
    wts = _f32_to_bf16_u16(wcat).view(ml_dtypes.bfloat16)
    b128 = np.concatenate([b, b]).reshape(128, 1).astype(np.float32)

    # ---- device ----------------------------------------------------------
    nc = _build_nc(classes, groups, S, N2)
    in_maps = [{"slots": bufs[c], "wts": wts, "b": b128} for c in range(NCORES)]
    trace = os.environ.get("KERNEL_TRACE", "0") == "1"
    kw_ = {}
    if trace:
        import tempfile
        kw_ = dict(trace=True, tmpdir=tempfile.mkdtemp(prefix="kernel_trace_"))
    res = run_bass_kernel_spmd(nc, in_maps, core_ids=list(range(NCORES)), **kw_)
    last_exec_ns = res.exec_time_ns

    # ---- scatter back ----------------------------------------------------
    outs = np.stack(
        [np.asarray(res.results[c]["out"], dtype=np.float32) for c in range(NCORES)]
    )                                                     # (8, 128, N2)
    outs = outs.reshape(NCORES, 2, D, N2)
    # per-(class, pair) output column (chunks relabel o0 in emission order)
    cls_base = np.concatenate(
        [[0], np.cumsum([m // 2 for (k, nk, m, sc, oc) in classes])]
    ).astype(np.int64)
    pair_ocol = np.zeros(int(cls_base[-1]), np.int64)
    for rank, k, c0, P, o0, eng, j0 in chunks:
        pair_ocol[cls_base[rank] + j0 : cls_base[rank] + j0 + P] = o0 + np.arange(P)
    real = counts > 0
    d_core = dom_core[real].astype(np.int64)
    d_ocol = pair_ocol[cls_base[dom_class[real]] + dom_pos[real] // 2]
    d_par = dom_pos[real] % 2
    full = np.empty((nd, D), np.float32)
    full[real] = outs[d_core, d_par, :, d_ocol]
    full[~real] = b
    return full
